# revision 44
# baseline (speedup 1.0000x reference)
"""BinaryConv2D Trainium2 kernel (v3: fp8 DoubleRow + pipelined input).

Reference computation:
    out = conv2d(sign(x), sign(w), SAME, stride 1)   # sign(v) = +1 if v>=0 else -1
    x: (64, 56, 56, 128) f32, w: (3, 3, 128, 256) f32 -> out (64, 56, 56, 256) f32

Strategy (data-parallel over batch, 8 images per NeuronCore):
  1. Input pipeline, chunked in row-bands (image 0 in quarters, rest in
     halves) so the tensor engine starts early and never starves:
     SWDGE cast-DMA f32 -> bf16 (HBM->HBM), HW xbar DMA-transpose
     (DRAM->SBUF) to channel-major, ACT Sign -> fp8e4 +-1 scattered into
     a zero-padded 58x58 plane per image (pads zeroed once via 3 small
     DVE memsets; interior overwritten per image).
  2. Weights binarized host-side into fp8 tap-pair blocks [ci, 2, co].
     9 taps = 4 DoubleRow pairs + 1 single per 128-wide co-half.
     DoubleRow contracts 2 taps x 128 ci per matmul; the moving operand
     is a custom 3D AP [ci, 2, 464] whose pair-dim stride is the
     tap-shift difference (overlapping reads are fine).  All values are
     +-1 so f32 PSUM accumulation is exact.
  3. Output window = PSUM [128 co_half, 464 px] (8 rows x 58).  PSUM is
     evacuated to per-image staging [128, 3136] fp16 (exact: |out| <=
     1152 < 2048), stripping pad columns; copies alternate between the
     Scalar and Vector engines (DVE 2-port copies would starve SWDGE
     descriptor generation early on).  One HWDGE DMA per (image,
     co-half) writes the co-major output [256, 8*3136] fp16.
  4. Host transposes [256, n, 56, 56] -> NHWC f32 (cheap numpy pass).
"""

import sys

if "/opt/trn_rl_repo" not in sys.path:
    sys.path.insert(0, "/opt/trn_rl_repo")

import numpy as np

import bass_rust
import concourse.bacc as bacc
import concourse.bass as bass
import concourse.mybir as mybir
from concourse.tile import TileContext
from concourse.bass_utils import run_bass_kernel_spmd

N_CORES = 8
IMGS = 8  # images per core
H = W = 56
C = 128  # input channels (= SBUF partitions)
O = 256  # output channels
PW = 58  # padded row width (cols 0 and 57 are the SAME-padding cols)
PH = 58  # padded rows (rows 0 and 57 are the SAME-padding rows)
PPI = PH * PW  # padded pixels per image (3364)
GUARD_L = 8
GUARD_R = 8
XP_LEN = GUARD_L + PPI + GUARD_R
NWIN = 7  # 8-output-row windows per image
NPX = 8 * PW  # window size (8 rows x 58 = 464 <= 512 psum-bank limit)
F32 = mybir.dt.float32
F16 = mybir.dt.float16
BF16 = mybir.dt.bfloat16
F8 = mybir.dt.float8e4
DR = mybir.MatmulPerfMode.DoubleRow


def shift(di, dj):
    return PW * (di - 1) + (dj - 1)


# 9 taps = 4 DoubleRow pairs + 1 single
PAIRS = [((0, 0), (0, 1)), ((0, 2), (1, 0)), ((1, 1), (1, 2)), ((2, 0), (2, 1))]
SINGLE = (2, 2)
PAIR_BASE = [shift(*a) for a, b in PAIRS]
PAIR_STRIDE = [shift(*b) - shift(*a) for a, b in PAIRS]
SINGLE_SHIFT = shift(*SINGLE)


def pair_ap(base_ap, pair_stride, n):
    """3D AP [128, 2, n]: [partition, pair(stride=pair_stride), col(stride 1)]."""
    ap = base_ap.copy()
    part = list(base_ap.ap[0])
    ap.ap = bass_rust.VecI64Pair([part, [pair_stride, 2], [1, n]])
    return ap


def build_nc() -> bass.Bass:
    nc = bacc.Bacc()
    x_t = nc.dram_tensor("x", [IMGS, H, W, C], F32, kind="ExternalInput")
    w_t = nc.dram_tensor("wall", [C, 9 * O], F8, kind="ExternalInput")
    wb_t = nc.dram_tensor("wbias", [128, 2], F32, kind="ExternalInput")
    y_t = nc.dram_tensor("out", [O, IMGS * H * W], F16, kind="ExternalOutput")
    xb_ts = [nc.dram_tensor(f"xb{i}", [H * W, C], BF16) for i in range(IMGS)]

    with TileContext(nc) as tc:
        with (
            tc.tile_pool(name="const", bufs=1) as constp,
            tc.tile_pool(name="xtr", bufs=3) as xtrp,
            tc.tile_pool(name="stage", bufs=6) as stagep,
            tc.tile_pool(name="psum", bufs=8, space="PSUM") as psump,
        ):
            wall = constp.tile([C, 9 * O], F8)
            nc.sync.dma_start(out=wall[:], in_=w_t[:])
            bias_s = constp.tile([128, 2], F32)
            nc.sync.dma_start(out=bias_s[:], in_=wb_t[:])

            def w_pair(p, h):  # [ci, 2, 128] view of pair p, co-half h
                off = (2 * p + h) * O
                return wall[:, off : off + O].rearrange("c (j o) -> c j o", j=2)

            def w_single(h):  # [ci, 128]
                off = 8 * O + h * (O // 2)
                return wall[:, off : off + O // 2]

            # persistent zero-padded fp8 planes, one per image; only the pad
            # cells are zeroed (3 small memsets) - the interior is fully
            # overwritten by the Sign scatter.
            xpads = []
            for i in range(IMGS):
                xp = constp.tile([C, XP_LEN], F8, tag=f"xpad{i}")
                # head: guards + top pad row + col0 of data row 1
                nc.vector.memset(xp[:, 0 : GUARD_L + PW + 1], 1.0)
                # interior: col57 of row r and col0 of row r+1 -> [58k-1, 58k+1)
                nc.vector.memset(
                    xp[
                        :, GUARD_L + 2 * PW - 1 : GUARD_L + 2 * PW - 1 + 55 * PW
                    ].rearrange("c (r w) -> c r w", w=PW)[:, :, 0:2],
                    1.0,
                )
                # tail: col57 of row 56 + bottom pad row + guards
                nc.vector.memset(xp[:, GUARD_L + 57 * PW - 1 : XP_LEN], 1.0)
                xpads.append(xp)

            # ---- input pipeline: cast -> transpose -> binarize, all images
            # in halves.  Each transpose->Sign link costs a fixed ~7us on the
            # Scalar FIFO, so fewer chunks beat finer ones: quartering image
            # 0 delayed image 1's input by ~14us without helping the head.
            def bands_of(i):
                return [(0, 24), (24, 32)] if i == 0 else [(0, 28), (28, 28)]

            # all casts first: GpSimd generates every descriptor before the
            # binarize ops occupy its Q7 cores
            for i in range(IMGS):
                for r0, nrows in bands_of(i):
                    a, b = r0 * W, (r0 + nrows) * W
                    nc.gpsimd.dma_start(
                        out=xb_ts[i][a:b],
                        in_=x_t[i].rearrange("h w c -> (h w) c")[a:b],
                    )
            # transposes alone on Scalar (pipeline freely); binarize on
            # GpSimd: x_enc = (x>=0)*2 in {0,2} fp8 - the -sum(w) bias is
            # folded into the PSUM evacuation, so results stay exact
            for i in range(IMGS):
                xtr = xtrp.tile([C, H * W], BF16)
                for r0, nrows in bands_of(i):
                    a, b = r0 * W, (r0 + nrows) * W
                    nc.scalar.dma_start(
                        out=xtr[:, a:b], in_=xb_ts[i][a:b], transpose=True
                    )
                    s0 = GUARD_L + (1 + r0) * PW + 1
                    dst = xpads[i][:, s0 : s0 + nrows * PW].rearrange(
                        "c (r w) -> c r w", w=PW
                    )[:, :, 0:W]
                    src = xtr[:, a:b].rearrange("c (r w) -> c r w", w=W)
                    nc.gpsimd.tensor_scalar(
                        dst, src, 0.0, 2.0,
                        op0=mybir.AluOpType.is_ge, op1=mybir.AluOpType.mult,
                    )

            # ---- conv: 7 windows x 2 co-halves x (4 DR + 1 single) ----
            for i in range(IMGS):
                stages = [
                    stagep.tile([C, H * W], F16, name=f"st{i}h{h}", tag="st")
                    for h in range(2)
                ]
                for win in range(NWIN):
                    q0 = GUARD_L + PW * (1 + 8 * win)
                    for h in range(2):
                        ps = psump.tile([128, NPX], F32)
                        for p in range(4):
                            a = q0 + PAIR_BASE[p]
                            rhs = pair_ap(
                                xpads[i][:, a : a + NPX], PAIR_STRIDE[p], NPX
                            )
                            nc.tensor.matmul(
                                ps[:],
                                w_pair(p, h),
                                rhs,
                                start=(p == 0),
                                stop=False,
                                perf_mode=DR,
                            )
                        a = q0 + SINGLE_SHIFT
                        nc.tensor.matmul(
                            ps[:],
                            w_single(h),
                            xpads[i][:, a : a + NPX],
                            start=False,
                            stop=True,
                        )
                        # strip pad cols during PSUM evacuation (f32 -> f16)
                        # on DVE; a pad-strip in the out-DMA AP instead makes
                        # the DMA 112-byte-run bound (~50 GB/s) - keep the
                        # stripping here and the DMA contiguous.
                        dst = stages[h][
                            :, win * 8 * W : (win + 1) * 8 * W
                        ].rearrange("c (r w) -> c r w", w=W)
                        src = ps[:].rearrange("c (r w) -> c r w", w=PW)[
                            :, :, 1 : 1 + W
                        ]
                        bb = bias_s[:, h : h + 1].copy()
                        bb.ap = bass_rust.VecI64Pair(
                            [list(bias_s[:, h : h + 1].ap[0]), [0, 8], [0, W]]
                        )
                        nc.vector.tensor_tensor(
                            dst, src, bb, op=mybir.AluOpType.subtract
                        )
                # Per-image out DMAs, alone on the Sync queue: they wait on
                # this image's copies, and only later out DMAs sit behind
                # them, so the wait poisons nothing.  Stage slots have ~3
                # images of reuse slack (bufs=6).  Last image: per-window
                # DMAs to shrink the kernel tail.
                if i < IMGS - 1:
                    for h in range(2):
                        nc.sync.dma_start(
                            out=y_t[
                                h * 128 : (h + 1) * 128,
                                i * H * W : (i + 1) * H * W,
                            ],
                            in_=stages[h][:],
                        )
                else:
                    for win in range(NWIN):
                        for h in range(2):
                            nc.sync.dma_start(
                                out=y_t[
                                    h * 128 : (h + 1) * 128,
                                    i * H * W + win * 8 * W : i * H * W
                                    + (win + 1) * 8 * W,
                                ],
                                in_=stages[h][
                                    :, win * 8 * W : (win + 1) * 8 * W
                                ],
                            )

    nc.finalize()
    return nc


_NC_CACHE = None


def _get_nc():
    global _NC_CACHE
    if _NC_CACHE is None:
        _NC_CACHE = build_nc()
    return _NC_CACHE


def prep_w(w: np.ndarray) -> np.ndarray:
    """Binarize + pack weights host-side: (3,3,128,256) f32 -> [128, 2304] fp8
    laid out as 4 pairs x 2 halves x [ci, 2tap, 128co] + 2 x [ci, 128co]."""
    import ml_dtypes

    wb = np.where(w >= 0, np.float32(1.0), np.float32(-1.0))  # [di,dj,ci,co]
    blocks = []
    for (diA, djA), (diB, djB) in PAIRS:
        for h in range(2):
            blk = np.stack(
                [
                    wb[diA, djA, :, h * 128 : (h + 1) * 128],
                    wb[diB, djB, :, h * 128 : (h + 1) * 128],
                ],
                axis=1,
            )  # [ci, 2, 128]
            blocks.append(blk.reshape(C, 256))
    di, dj = SINGLE
    for h in range(2):
        blocks.append(wb[di, dj, :, h * 128 : (h + 1) * 128])  # [ci, 128]
    wall = np.concatenate(blocks, axis=1)  # [128, 2304]
    assert wall.shape == (C, 9 * O)
    bias = wb.sum(axis=(0, 1, 2)).astype(np.float32)  # [256]
    wbias = np.ascontiguousarray(bias.reshape(2, 128).T)  # [128, 2]
    return np.ascontiguousarray(wall.astype(ml_dtypes.float8_e4m3)), wbias


def _ntff_hook():
    sys.path.insert(0, "/root/.axon_site")
    from trn_agent_boot.trn_boot import _ntff_profile_via_ctypes

    return _ntff_profile_via_ctypes("/opt/axon/libaxon_pjrt.so")


def run(inputs: dict, profile_dir: str | None = None):
    """Run on all 8 NeuronCores. Returns (full_output, BassKernelResults)."""
    x = np.ascontiguousarray(np.asarray(inputs["x"], dtype=np.float32))
    w = np.ascontiguousarray(np.asarray(inputs["w"], dtype=np.float32))
    assert x.shape == (N_CORES * IMGS, H, W, C), x.shape
    assert w.shape == (3, 3, C, O), w.shape

    nc = _get_nc()
    wall, wbias = prep_w(w)
    in_maps = [
        {"x": x[i * IMGS : (i + 1) * IMGS], "wall": wall, "wbias": wbias}
        for i in range(N_CORES)
    ]
    if profile_dir is not None:
        hook = _ntff_hook()
        with hook(profile_dir, [0]):
            res = run_bass_kernel_spmd(nc, in_maps, list(range(N_CORES)))
    else:
        res = run_bass_kernel_spmd(nc, in_maps, list(range(N_CORES)))

    out = np.empty((N_CORES * IMGS, H, W, O), dtype=np.float32)
    for i in range(N_CORES):
        yc = np.asarray(res.results[i]["out"])  # [256, 8*3136] fp16
        out[i * IMGS : (i + 1) * IMGS] = (
            yc.astype(np.float32).reshape(O, IMGS, H, W).transpose(1, 2, 3, 0)
        )
    return out, res


def kernel(**inputs: np.ndarray) -> np.ndarray:
    out, _ = run(inputs)
    return out


# revision 46
# speedup vs baseline: 2.2714x; 2.2714x over previous
"""BinaryConv2D Trainium2 kernel (v3: fp8 DoubleRow + pipelined input).

Reference computation:
    out = conv2d(sign(x), sign(w), SAME, stride 1)   # sign(v) = +1 if v>=0 else -1
    x: (64, 56, 56, 128) f32, w: (3, 3, 128, 256) f32 -> out (64, 56, 56, 256) f32

Strategy (data-parallel over batch, 8 images per NeuronCore):
  1. Input pipeline, chunked in row-bands (image 0 in quarters, rest in
     halves) so the tensor engine starts early and never starves:
     SWDGE cast-DMA f32 -> bf16 (HBM->HBM), HW xbar DMA-transpose
     (DRAM->SBUF) to channel-major, ACT Sign -> fp8e4 +-1 scattered into
     a zero-padded 58x58 plane per image (pads zeroed once via 3 small
     DVE memsets; interior overwritten per image).
  2. Weights binarized host-side into fp8 tap-pair blocks [ci, 2, co].
     9 taps = 4 DoubleRow pairs + 1 single per 128-wide co-half.
     DoubleRow contracts 2 taps x 128 ci per matmul; the moving operand
     is a custom 3D AP [ci, 2, 464] whose pair-dim stride is the
     tap-shift difference (overlapping reads are fine).  All values are
     +-1 so f32 PSUM accumulation is exact.
  3. Output window = PSUM [128 co_half, 464 px] (8 rows x 58).  PSUM is
     evacuated to per-image staging [128, 3136] fp16 (exact: |out| <=
     1152 < 2048), stripping pad columns; copies alternate between the
     Scalar and Vector engines (DVE 2-port copies would starve SWDGE
     descriptor generation early on).  One HWDGE DMA per (image,
     co-half) writes the co-major output [256, 8*3136] fp16.
  4. Host transposes [256, n, 56, 56] -> NHWC f32 (cheap numpy pass).
"""

import sys

if "/opt/trn_rl_repo" not in sys.path:
    sys.path.insert(0, "/opt/trn_rl_repo")

import numpy as np

import bass_rust
import concourse.bacc as bacc
import concourse.bass as bass
import concourse.mybir as mybir
from concourse.tile import TileContext
from concourse.bass_utils import run_bass_kernel_spmd

N_CORES = 8
IMGS = 8  # images per core
H = W = 56
C = 128  # input channels (= SBUF partitions)
O = 256  # output channels
PW = 58  # padded row width (cols 0 and 57 are the SAME-padding cols)
PH = 58  # padded rows (rows 0 and 57 are the SAME-padding rows)
PPI = PH * PW  # padded pixels per image (3364)
GUARD_L = 8
GUARD_R = 8
XP_LEN = GUARD_L + PPI + GUARD_R
NWIN = 7  # 8-output-row windows per image
NPX = 8 * PW  # window size (8 rows x 58 = 464 <= 512 psum-bank limit)
F32 = mybir.dt.float32
F16 = mybir.dt.float16
BF16 = mybir.dt.bfloat16
F8 = mybir.dt.float8e4
DR = mybir.MatmulPerfMode.DoubleRow


def shift(di, dj):
    return PW * (di - 1) + (dj - 1)


# 9 taps = 4 DoubleRow pairs + 1 single
PAIRS = [((0, 0), (0, 1)), ((0, 2), (1, 0)), ((1, 1), (1, 2)), ((2, 0), (2, 1))]
SINGLE = (2, 2)
PAIR_BASE = [shift(*a) for a, b in PAIRS]
PAIR_STRIDE = [shift(*b) - shift(*a) for a, b in PAIRS]
SINGLE_SHIFT = shift(*SINGLE)


def pair_ap(base_ap, pair_stride, n):
    """3D AP [128, 2, n]: [partition, pair(stride=pair_stride), col(stride 1)]."""
    ap = base_ap.copy()
    part = list(base_ap.ap[0])
    ap.ap = bass_rust.VecI64Pair([part, [pair_stride, 2], [1, n]])
    return ap


def build_nc() -> bass.Bass:
    nc = bacc.Bacc()
    x_t = nc.dram_tensor("x", [IMGS, H, W, C], F32, kind="ExternalInput")
    w_t = nc.dram_tensor("wall", [C, 9 * O], F8, kind="ExternalInput")
    wb_t = nc.dram_tensor("wbias", [128, 2], F32, kind="ExternalInput")
    y_t = nc.dram_tensor("out", [O, IMGS * H * W], F16, kind="ExternalOutput")
    xb_ts = [nc.dram_tensor(f"xb{i}", [H * W, C], BF16) for i in range(IMGS)]

    with TileContext(nc) as tc:
        with (
            tc.tile_pool(name="const", bufs=1) as constp,
            tc.tile_pool(name="xtr", bufs=3) as xtrp,
            tc.tile_pool(name="stage", bufs=6) as stagep,
            tc.tile_pool(name="psum", bufs=8, space="PSUM") as psump,
        ):
            wall = constp.tile([C, 9 * O], F8)
            nc.sync.dma_start(out=wall[:], in_=w_t[:])
            bias_s = constp.tile([128, 2], F32)
            nc.sync.dma_start(out=bias_s[:], in_=wb_t[:])

            def w_pair(p, h):  # [ci, 2, 128] view of pair p, co-half h
                off = (2 * p + h) * O
                return wall[:, off : off + O].rearrange("c (j o) -> c j o", j=2)

            def w_single(h):  # [ci, 128]
                off = 8 * O + h * (O // 2)
                return wall[:, off : off + O // 2]

            # persistent zero-padded fp8 planes, one per image; only the pad
            # cells are zeroed (3 small memsets) - the interior is fully
            # overwritten by the Sign scatter.
            xpads = []
            for i in range(IMGS):
                xp = constp.tile([C, XP_LEN], F8, tag=f"xpad{i}")
                # head: guards + top pad row + col0 of data row 1
                nc.vector.memset(xp[:, 0 : GUARD_L + PW + 1], 1.0)
                # interior: col57 of row r and col0 of row r+1 -> [58k-1, 58k+1)
                nc.vector.memset(
                    xp[
                        :, GUARD_L + 2 * PW - 1 : GUARD_L + 2 * PW - 1 + 55 * PW
                    ].rearrange("c (r w) -> c r w", w=PW)[:, :, 0:2],
                    1.0,
                )
                # tail: col57 of row 56 + bottom pad row + guards
                nc.vector.memset(xp[:, GUARD_L + 57 * PW - 1 : XP_LEN], 1.0)
                xpads.append(xp)

            # ---- input pipeline: cast -> transpose -> binarize, all images
            # in halves.  Each transpose->Sign link costs a fixed ~7us on the
            # Scalar FIFO, so fewer chunks beat finer ones: quartering image
            # 0 delayed image 1's input by ~14us without helping the head.
            for i in range(IMGS):
                # image 0: 24+32 rows - the small first chunk covers windows
                # 0-1 so the matmul stream starts ~7us earlier; same 2 links
                bands = [(0, 24), (24, 32)] if i == 0 else [(0, 28), (28, 28)]
                xtr = xtrp.tile([C, H * W], BF16)
                for r0, nrows in bands:
                    a, b = r0 * W, (r0 + nrows) * W
                    nc.gpsimd.dma_start(
                        out=xb_ts[i][a:b],
                        in_=x_t[i].rearrange("h w c -> (h w) c")[a:b],
                    )
                    nc.scalar.dma_start(
                        out=xtr[:, a:b], in_=xb_ts[i][a:b], transpose=True
                    )
                    s0 = GUARD_L + (1 + r0) * PW + 1
                    dst = xpads[i][:, s0 : s0 + nrows * PW].rearrange(
                        "c (r w) -> c r w", w=PW
                    )[:, :, 0:W]
                    src = xtr[:, a:b].rearrange("c (r w) -> c r w", w=W)
                    # binarize on DVE: x_enc = (x>=0)*2 in {0,2} fp8; the
                    # -sum(w) bias is folded into the PSUM evacuation.  This
                    # leaves the Scalar queue with transposes only, so the
                    # input chain is no longer Sign-serialized.
                    nc.vector.tensor_scalar(
                        dst, src, 0.0, 2.0,
                        op0=mybir.AluOpType.is_ge, op1=mybir.AluOpType.mult,
                    )

            # ---- conv: 7 windows x 2 co-halves x (4 DR + 1 single) ----
            for i in range(IMGS):
                stages = [
                    stagep.tile([C, H * W], F16, name=f"st{i}h{h}", tag="st")
                    for h in range(2)
                ]
                for win in range(NWIN):
                    q0 = GUARD_L + PW * (1 + 8 * win)
                    for h in range(2):
                        ps = psump.tile([128, NPX], F32)
                        for p in range(4):
                            a = q0 + PAIR_BASE[p]
                            rhs = pair_ap(
                                xpads[i][:, a : a + NPX], PAIR_STRIDE[p], NPX
                            )
                            nc.tensor.matmul(
                                ps[:],
                                w_pair(p, h),
                                rhs,
                                start=(p == 0),
                                stop=False,
                                perf_mode=DR,
                            )
                        a = q0 + SINGLE_SHIFT
                        nc.tensor.matmul(
                            ps[:],
                            w_single(h),
                            xpads[i][:, a : a + NPX],
                            start=False,
                            stop=True,
                        )
                        # strip pad cols during PSUM evacuation (f32 -> f16)
                        # on DVE; a pad-strip in the out-DMA AP instead makes
                        # the DMA 112-byte-run bound (~50 GB/s) - keep the
                        # stripping here and the DMA contiguous.
                        dst = stages[h][
                            :, win * 8 * W : (win + 1) * 8 * W
                        ].rearrange("c (r w) -> c r w", w=W)
                        src = ps[:].rearrange("c (r w) -> c r w", w=PW)[
                            :, :, 1 : 1 + W
                        ]
                        bb = bias_s[:, h : h + 1].copy()
                        bb.ap = bass_rust.VecI64Pair(
                            [list(bias_s[:, h : h + 1].ap[0]), [0, 8], [0, W]]
                        )
                        nc.vector.tensor_tensor(
                            dst, src, bb, op=mybir.AluOpType.subtract
                        )
                # Per-image out DMAs, alone on the Sync queue: they wait on
                # this image's copies, and only later out DMAs sit behind
                # them, so the wait poisons nothing.  Stage slots have ~3
                # images of reuse slack (bufs=6).  Last image: per-window
                # DMAs to shrink the kernel tail.
                if i < IMGS - 1:
                    for h in range(2):
                        nc.sync.dma_start(
                            out=y_t[
                                h * 128 : (h + 1) * 128,
                                i * H * W : (i + 1) * H * W,
                            ],
                            in_=stages[h][:],
                        )
                else:
                    for win in range(NWIN):
                        for h in range(2):
                            nc.sync.dma_start(
                                out=y_t[
                                    h * 128 : (h + 1) * 128,
                                    i * H * W + win * 8 * W : i * H * W
                                    + (win + 1) * 8 * W,
                                ],
                                in_=stages[h][
                                    :, win * 8 * W : (win + 1) * 8 * W
                                ],
                            )

    nc.finalize()
    return nc


_NC_CACHE = None


def _get_nc():
    global _NC_CACHE
    if _NC_CACHE is None:
        _NC_CACHE = build_nc()
    return _NC_CACHE


def prep_w(w: np.ndarray) -> np.ndarray:
    """Binarize + pack weights host-side: (3,3,128,256) f32 -> [128, 2304] fp8
    laid out as 4 pairs x 2 halves x [ci, 2tap, 128co] + 2 x [ci, 128co]."""
    import ml_dtypes

    wb = np.where(w >= 0, np.float32(1.0), np.float32(-1.0))  # [di,dj,ci,co]
    blocks = []
    for (diA, djA), (diB, djB) in PAIRS:
        for h in range(2):
            blk = np.stack(
                [
                    wb[diA, djA, :, h * 128 : (h + 1) * 128],
                    wb[diB, djB, :, h * 128 : (h + 1) * 128],
                ],
                axis=1,
            )  # [ci, 2, 128]
            blocks.append(blk.reshape(C, 256))
    di, dj = SINGLE
    for h in range(2):
        blocks.append(wb[di, dj, :, h * 128 : (h + 1) * 128])  # [ci, 128]
    wall = np.concatenate(blocks, axis=1)  # [128, 2304]
    assert wall.shape == (C, 9 * O)
    bias = wb.sum(axis=(0, 1, 2)).astype(np.float32)  # [256]
    wbias = np.ascontiguousarray(bias.reshape(2, 128).T)  # [128, 2]
    return np.ascontiguousarray(wall.astype(ml_dtypes.float8_e4m3)), wbias


def _ntff_hook():
    sys.path.insert(0, "/root/.axon_site")
    from trn_agent_boot.trn_boot import _ntff_profile_via_ctypes

    return _ntff_profile_via_ctypes("/opt/axon/libaxon_pjrt.so")


def run(inputs: dict, profile_dir: str | None = None):
    """Run on all 8 NeuronCores. Returns (full_output, BassKernelResults)."""
    x = np.ascontiguousarray(np.asarray(inputs["x"], dtype=np.float32))
    w = np.ascontiguousarray(np.asarray(inputs["w"], dtype=np.float32))
    assert x.shape == (N_CORES * IMGS, H, W, C), x.shape
    assert w.shape == (3, 3, C, O), w.shape

    nc = _get_nc()
    wall, wbias = prep_w(w)
    in_maps = [
        {"x": x[i * IMGS : (i + 1) * IMGS], "wall": wall, "wbias": wbias}
        for i in range(N_CORES)
    ]
    if profile_dir is not None:
        hook = _ntff_hook()
        with hook(profile_dir, [0]):
            res = run_bass_kernel_spmd(nc, in_maps, list(range(N_CORES)))
    else:
        res = run_bass_kernel_spmd(nc, in_maps, list(range(N_CORES)))

    out = np.empty((N_CORES * IMGS, H, W, O), dtype=np.float32)
    for i in range(N_CORES):
        yc = np.asarray(res.results[i]["out"])  # [256, 8*3136] fp16
        out[i * IMGS : (i + 1) * IMGS] = (
            yc.astype(np.float32).reshape(O, IMGS, H, W).transpose(1, 2, 3, 0)
        )
    return out, res


def kernel(**inputs: np.ndarray) -> np.ndarray:
    out, _ = run(inputs)
    return out


# revision 48
# speedup vs baseline: 2.8366x; 1.2488x over previous
"""BinaryConv2D Trainium2 kernel (v3: fp8 DoubleRow + pipelined input).

Reference computation:
    out = conv2d(sign(x), sign(w), SAME, stride 1)   # sign(v) = +1 if v>=0 else -1
    x: (64, 56, 56, 128) f32, w: (3, 3, 128, 256) f32 -> out (64, 56, 56, 256) f32

Strategy (data-parallel over batch, 8 images per NeuronCore):
  1. Input pipeline, chunked in row-bands (image 0 in quarters, rest in
     halves) so the tensor engine starts early and never starves:
     SWDGE cast-DMA f32 -> bf16 (HBM->HBM), HW xbar DMA-transpose
     (DRAM->SBUF) to channel-major, ACT Sign -> fp8e4 +-1 scattered into
     a zero-padded 58x58 plane per image (pads zeroed once via 3 small
     DVE memsets; interior overwritten per image).
  2. Weights binarized host-side into fp8 tap-pair blocks [ci, 2, co].
     9 taps = 4 DoubleRow pairs + 1 single per 128-wide co-half.
     DoubleRow contracts 2 taps x 128 ci per matmul; the moving operand
     is a custom 3D AP [ci, 2, 464] whose pair-dim stride is the
     tap-shift difference (overlapping reads are fine).  All values are
     +-1 so f32 PSUM accumulation is exact.
  3. Output window = PSUM [128 co_half, 464 px] (8 rows x 58).  PSUM is
     evacuated to per-image staging [128, 3136] fp16 (exact: |out| <=
     1152 < 2048), stripping pad columns; copies alternate between the
     Scalar and Vector engines (DVE 2-port copies would starve SWDGE
     descriptor generation early on).  One HWDGE DMA per (image,
     co-half) writes the co-major output [256, 8*3136] fp16.
  4. Host transposes [256, n, 56, 56] -> NHWC f32 (cheap numpy pass).
"""

import sys

if "/opt/trn_rl_repo" not in sys.path:
    sys.path.insert(0, "/opt/trn_rl_repo")

import numpy as np

import bass_rust
import concourse.bacc as bacc
import concourse.bass as bass
import concourse.mybir as mybir
from concourse.tile import TileContext
from concourse.bass_utils import run_bass_kernel_spmd

N_CORES = 8
IMGS = 8  # images per core
H = W = 56
C = 128  # input channels (= SBUF partitions)
O = 256  # output channels
PW = 58  # padded row width (cols 0 and 57 are the SAME-padding cols)
PH = 58  # padded rows (rows 0 and 57 are the SAME-padding rows)
PPI = PH * PW  # padded pixels per image (3364)
GUARD_L = 8
GUARD_R = 8
XP_LEN = GUARD_L + PPI + GUARD_R
NWIN = 7  # 8-output-row windows per image
NPX = 8 * PW  # window size (8 rows x 58 = 464 <= 512 psum-bank limit)
F32 = mybir.dt.float32
F16 = mybir.dt.float16
BF16 = mybir.dt.bfloat16
F8 = mybir.dt.float8e4
DR = mybir.MatmulPerfMode.DoubleRow


def shift(di, dj):
    return PW * (di - 1) + (dj - 1)


# 9 taps = 4 DoubleRow pairs + 1 single
PAIRS = [((0, 0), (0, 1)), ((0, 2), (1, 0)), ((1, 1), (1, 2)), ((2, 0), (2, 1))]
SINGLE = (2, 2)
PAIR_BASE = [shift(*a) for a, b in PAIRS]
PAIR_STRIDE = [shift(*b) - shift(*a) for a, b in PAIRS]
SINGLE_SHIFT = shift(*SINGLE)


def pair_ap(base_ap, pair_stride, n):
    """3D AP [128, 2, n]: [partition, pair(stride=pair_stride), col(stride 1)]."""
    ap = base_ap.copy()
    part = list(base_ap.ap[0])
    ap.ap = bass_rust.VecI64Pair([part, [pair_stride, 2], [1, n]])
    return ap


def build_nc() -> bass.Bass:
    nc = bacc.Bacc()
    x_t = nc.dram_tensor("x", [IMGS, H, W, C], F32, kind="ExternalInput")
    w_t = nc.dram_tensor("wall", [C, 9 * O], F8, kind="ExternalInput")
    y_t = nc.dram_tensor("out", [O, IMGS * H * W], F16, kind="ExternalOutput")
    xb_ts = [nc.dram_tensor(f"xb{i}", [H * W, C], BF16) for i in range(IMGS)]

    with TileContext(nc) as tc:
        with (
            tc.tile_pool(name="const", bufs=1) as constp,
            tc.tile_pool(name="xtr", bufs=4) as xtrp,
            tc.tile_pool(name="stage", bufs=8) as stagep,
            tc.tile_pool(name="psum", bufs=8, space="PSUM") as psump,
        ):
            wall = constp.tile([C, 9 * O], F8)
            nc.sync.dma_start(out=wall[:], in_=w_t[:])

            def w_pair(p, h):  # [ci, 2, 128] view of pair p, co-half h
                off = (2 * p + h) * O
                return wall[:, off : off + O].rearrange("c (j o) -> c j o", j=2)

            def w_single(h):  # [ci, 128]
                off = 8 * O + h * (O // 2)
                return wall[:, off : off + O // 2]

            # persistent zero-padded fp8 planes, one per image; only the pad
            # cells are zeroed (3 small memsets) - the interior is fully
            # overwritten by the Sign scatter.
            xpads = []
            for i in range(IMGS):
                xp = constp.tile([C, XP_LEN], F8, tag=f"xpad{i}")
                # head: guards + top pad row + col0 of data row 1
                nc.vector.memset(xp[:, 0 : GUARD_L + PW + 1], 0.0)
                # interior: col57 of row r and col0 of row r+1 -> [58k-1, 58k+1)
                nc.vector.memset(
                    xp[
                        :, GUARD_L + 2 * PW - 1 : GUARD_L + 2 * PW - 1 + 55 * PW
                    ].rearrange("c (r w) -> c r w", w=PW)[:, :, 0:2],
                    0.0,
                )
                # tail: col57 of row 56 + bottom pad row + guards
                nc.vector.memset(xp[:, GUARD_L + 57 * PW - 1 : XP_LEN], 0.0)
                xpads.append(xp)

            # ---- input pipeline: cast -> transpose -> binarize, all images
            # in halves.  Each transpose->Sign link costs a fixed ~7us on the
            # Scalar FIFO, so fewer chunks beat finer ones: quartering image
            # 0 delayed image 1's input by ~14us without helping the head.
            for i in range(IMGS):
                # image 0: 24+32 rows - the small first chunk covers windows
                # 0-1 so the matmul stream starts ~7us earlier; same 2 links
                bands = [(0, 24), (24, 32)] if i == 0 else [(0, 28), (28, 28)]
                xtr = xtrp.tile([C, H * W], BF16)
                for r0, nrows in bands:
                    a, b = r0 * W, (r0 + nrows) * W
                    nc.gpsimd.dma_start(
                        out=xb_ts[i][a:b],
                        in_=x_t[i].rearrange("h w c -> (h w) c")[a:b],
                    )
                    nc.scalar.dma_start(
                        out=xtr[:, a:b], in_=xb_ts[i][a:b], transpose=True
                    )
                    s0 = GUARD_L + (1 + r0) * PW + 1
                    dst = xpads[i][:, s0 : s0 + nrows * PW].rearrange(
                        "c (r w) -> c r w", w=PW
                    )[:, :, 0:W]
                    src = xtr[:, a:b].rearrange("c (r w) -> c r w", w=W)
                    nc.scalar.activation(
                        dst, src, mybir.ActivationFunctionType.Sign
                    )

            # ---- conv: 7 windows x 2 co-halves x (4 DR + 1 single) ----
            for i in range(IMGS):
                stages = [
                    stagep.tile([C, H * W], F16, name=f"st{i}h{h}", tag="st")
                    for h in range(2)
                ]
                for win in range(NWIN):
                    q0 = GUARD_L + PW * (1 + 8 * win)
                    for h in range(2):
                        ps = psump.tile([128, NPX], F32)
                        for p in range(4):
                            a = q0 + PAIR_BASE[p]
                            rhs = pair_ap(
                                xpads[i][:, a : a + NPX], PAIR_STRIDE[p], NPX
                            )
                            nc.tensor.matmul(
                                ps[:],
                                w_pair(p, h),
                                rhs,
                                start=(p == 0),
                                stop=False,
                                perf_mode=DR,
                            )
                        a = q0 + SINGLE_SHIFT
                        nc.tensor.matmul(
                            ps[:],
                            w_single(h),
                            xpads[i][:, a : a + NPX],
                            start=False,
                            stop=True,
                        )
                        # strip pad cols during PSUM evacuation (f32 -> f16)
                        # on DVE; a pad-strip in the out-DMA AP instead makes
                        # the DMA 112-byte-run bound (~50 GB/s) - keep the
                        # stripping here and the DMA contiguous.
                        dst = stages[h][
                            :, win * 8 * W : (win + 1) * 8 * W
                        ].rearrange("c (r w) -> c r w", w=W)
                        src = ps[:].rearrange("c (r w) -> c r w", w=PW)[
                            :, :, 1 : 1 + W
                        ]
                        nc.vector.tensor_copy(dst, src)
                # Per-image out DMAs, alone on the Sync queue: they wait on
                # this image's copies, and only later out DMAs sit behind
                # them, so the wait poisons nothing.  Stage slots have ~3
                # images of reuse slack (bufs=6).  Last image: per-window
                # DMAs to shrink the kernel tail.
                if i < IMGS - 1:
                    for h in range(2):
                        nc.sync.dma_start(
                            out=y_t[
                                h * 128 : (h + 1) * 128,
                                i * H * W : (i + 1) * H * W,
                            ],
                            in_=stages[h][:],
                        )
                else:
                    for win in range(NWIN):
                        for h in range(2):
                            eng = nc.sync if h else nc.scalar
                            eng.dma_start(
                                out=y_t[
                                    h * 128 : (h + 1) * 128,
                                    i * H * W + win * 8 * W : i * H * W
                                    + (win + 1) * 8 * W,
                                ],
                                in_=stages[h][
                                    :, win * 8 * W : (win + 1) * 8 * W
                                ],
                            )

    nc.finalize()
    return nc


_NC_CACHE = None


def _get_nc():
    global _NC_CACHE
    if _NC_CACHE is None:
        _NC_CACHE = build_nc()
    return _NC_CACHE


def prep_w(w: np.ndarray) -> np.ndarray:
    """Binarize + pack weights host-side: (3,3,128,256) f32 -> [128, 2304] fp8
    laid out as 4 pairs x 2 halves x [ci, 2tap, 128co] + 2 x [ci, 128co]."""
    import ml_dtypes

    wb = np.where(w >= 0, np.float32(1.0), np.float32(-1.0))  # [di,dj,ci,co]
    blocks = []
    for (diA, djA), (diB, djB) in PAIRS:
        for h in range(2):
            blk = np.stack(
                [
                    wb[diA, djA, :, h * 128 : (h + 1) * 128],
                    wb[diB, djB, :, h * 128 : (h + 1) * 128],
                ],
                axis=1,
            )  # [ci, 2, 128]
            blocks.append(blk.reshape(C, 256))
    di, dj = SINGLE
    for h in range(2):
        blocks.append(wb[di, dj, :, h * 128 : (h + 1) * 128])  # [ci, 128]
    wall = np.concatenate(blocks, axis=1)  # [128, 2304]
    assert wall.shape == (C, 9 * O)
    return np.ascontiguousarray(wall.astype(ml_dtypes.float8_e4m3))


def _ntff_hook():
    sys.path.insert(0, "/root/.axon_site")
    from trn_agent_boot.trn_boot import _ntff_profile_via_ctypes

    return _ntff_profile_via_ctypes("/opt/axon/libaxon_pjrt.so")


def run(inputs: dict, profile_dir: str | None = None):
    """Run on all 8 NeuronCores. Returns (full_output, BassKernelResults)."""
    x = np.ascontiguousarray(np.asarray(inputs["x"], dtype=np.float32))
    w = np.ascontiguousarray(np.asarray(inputs["w"], dtype=np.float32))
    assert x.shape == (N_CORES * IMGS, H, W, C), x.shape
    assert w.shape == (3, 3, C, O), w.shape

    nc = _get_nc()
    wall = prep_w(w)
    in_maps = [
        {"x": x[i * IMGS : (i + 1) * IMGS], "wall": wall} for i in range(N_CORES)
    ]
    if profile_dir is not None:
        hook = _ntff_hook()
        with hook(profile_dir, [0]):
            res = run_bass_kernel_spmd(nc, in_maps, list(range(N_CORES)))
    else:
        res = run_bass_kernel_spmd(nc, in_maps, list(range(N_CORES)))

    out = np.empty((N_CORES * IMGS, H, W, O), dtype=np.float32)
    for i in range(N_CORES):
        yc = np.asarray(res.results[i]["out"])  # [256, 8*3136] fp16
        out[i * IMGS : (i + 1) * IMGS] = (
            yc.astype(np.float32).reshape(O, IMGS, H, W).transpose(1, 2, 3, 0)
        )
    return out, res


def kernel(**inputs: np.ndarray) -> np.ndarray:
    out, _ = run(inputs)
    return out
